# revision 68
# baseline (speedup 1.0000x reference)
"""Fused GQA attention block (QKV proj + RoPE + SDPA + out proj) on 8 TRN2
NeuronCores.

Sharding: tensor-parallel over heads. Core c owns kv-head c (q-heads
4c..4c+3): Wq/Wk/Wv column shards, Wo row shard. Each core computes a
full-shape partial of the output projection; the host sums the 8 partials.

All matmul operands are bf16 (PSUM accumulation stays fp32; measured rel
err ~6e-3 vs the 2e-2 gate): halves DMA traffic and SBUF footprint at
unchanged PE speed (1 cycle/row).

Per-core dataflow:
  phase 1: Q^T/K^T/V^T = W^T X^T accumulated over D, two passes of 3
           output chunks (3+3 PSUM banks); X tile and all weights
           SBUF-resident (X chunk-streamed one token-tile ahead).  Each
           pass's epilogue (RoPE / V-transpose) is deferred into the next
           pass's matmul stream so the PE never waits on PSUM drains:
           RoPE's rotate-half runs as two SBUF->SBUF DMA partition moves
           (sign folded into the host-built signed-sin table) + two muls
           and an add on DVE; V^T re-transposed to natural [token, hd]
           chunks on the PE.  The final epilogue's V-transposes defer
           into phase 2 under the first scores.
  phase 2: per (batch, q-head): S^T = K^T.T @ Q^T; P^T = exp(S^T*scale)
           on ACT straight out of PSUM (bf16 out); O^T = V.T @ P^T.
           Scores issue LA k-chunks ahead of the P@V accumulation; each
           head's finalize (denominator matmul -> reciprocal ->
           normalize) defers into the next head's prologue so its DVE
           tail hides under PE work.  Denominators: group 0 sums exp
           chunks by per-chunk ones-matmuls (free PE slots - it hosts no
           out-proj and is exp-paced); later groups use a 7-add bf16 DVE
           tree + one 512-row ones-matmul.  No row-max subtraction:
           scores are O(5), exp cannot overflow.
  phase 3: out_partial = O^T.T @ Wo shard (Wo SBUF-resident), one
           (token-chunk, 512-col) chunk interleaved into each phase-2
           kc-slot (the 4-matmul chunk keeps the PE busy while ACT runs
           the exps); stores stream to DRAM in bf16, fine-grained for the
           trailing group so the final drain is short.
"""

from contextlib import ExitStack

import numpy as np

B, S, D = 2, 1024, 4096
HQ, HKV, HD = 32, 8, 128
NCORES = 8
QH = HQ // NCORES          # 4 q heads per core
MQ = QH * HD               # 512 q-projection columns per core
TT = B * S                 # 2048 tokens
P = 128
T5 = 512                   # token macro-tile
NT5 = TT // T5             # 4
ND = D // P                # 32 contraction chunks
NKC = S // P               # 8 key chunks per batch
SCALE = HD ** -0.5

_CACHE = {}


def _build_kernel(tc, out_ap, ins):
    from concourse import mybir

    nc = tc.nc
    F32 = mybir.dt.float32
    BF16 = mybir.dt.bfloat16
    FP32R = mybir.dt.float32r
    Exp = mybir.ActivationFunctionType.Exp

    hst, cs_d, wq, wkv, wo, cbf = ins

    ctx = tc.ctx  # set by caller
    const = ctx.enter_context(tc.tile_pool(name="const", bufs=1))
    persist = ctx.enter_context(tc.tile_pool(name="persist", bufs=1))
    # ropep/tp stay open for the whole kernel: if they closed with phase 1,
    # phase 2's pools would allocate over their released zones and wait for
    # the final RoPE epilogue's DMA chains before the first exp; keeping tp
    # open also lets the final V-transposes defer into phase 2
    ropep = ctx.enter_context(tc.tile_pool(name="ropep", bufs=2))
    tp_psum = ctx.enter_context(tc.tile_pool(name="tp_ps", bufs=1,
                                             space="PSUM"))

    # ---- constants: identity+ones (bf16) --------------------------------
    # (tile allocated here; its DMA is emitted after the startup-critical
    # X/weight chunks so the first matmuls aren't queued behind it)
    cc = const.tile([P, 2, P], BF16)
    ident = cc[:, 0]
    ones = cc[:, 1]
    # ---- persistent activations (all bf16) ------------------------------
    qT = persist.tile([P, QH, TT], BF16)       # Q^T per head
    kT = persist.tile([P, TT], BF16)           # K^T (one kv head)
    vN = persist.tile([P, TT // P, P], BF16)   # V natural [tok, hd] chunks
    oT = persist.tile([P, QH, TT], BF16)       # attention out, transposed

    # ---- phases 0+1: projections + RoPE ---------------------------------
    # Two passes of 3 output chunks per token tile (3+3 PSUM banks), with
    # each pass's RoPE/transpose epilogue deferred into the next pass's
    # matmul stream so the PE never waits on PSUM drains or DVE chains.
    wq_r = wq.rearrange("(o p) m -> p o m", p=P)    # [128, 32, 512]
    wkv_r = wkv.rearrange("(o p) m -> p o m", p=P)  # [128, 32, 256]
    hst_r = hst.rearrange("(o p) t -> p o t", p=P)  # [128, 32, 2048]
    OCS_A, OCS_B = (QH, 0, 1), (QH + 1, 2, 3)       # K,Q0,Q1 | V,Q2,Q3
    # (V leads pass B's epilogue so its PSUM bank + the transpose bank drain
    # early; phase 2's pools then land on banks that free first.)

    with tc.tile_pool(name="ph1", bufs=1) as ph1, \
         tc.tile_pool(name="xpool", bufs=2) as xpool, \
         tc.tile_pool(name="psA", bufs=3, space="PSUM") as psA, \
         tc.tile_pool(name="psB", bufs=3, space="PSUM") as psB:
        wq_res = ph1.tile([P, ND, MQ], BF16)    # resident Wq shard (4 MB)
        wkv_res = ph1.tile([P, ND, 2 * HD], BF16)
        # cos/sin lives in the ctx-scoped pool: the final epilogue's rope
        # muls read it after phase 1's pools close; if it sat in ph1, the
        # ph1 zone release (and with it phase 2's pool opens, exps, and
        # first l-matmul) would wait for those muls
        cs_res = ropep.tile([P, 2, TT], F32, tag="cs", bufs=1, name="cs_res")

        def load_x(t5):
            # 4 chunked DMAs: pass A's first matmuls wait only on chunk 0,
            # and other queue traffic can interleave between chunks
            xres = xpool.tile([P, ND, T5], BF16, tag="x", name="xres")
            tsl = slice(t5 * T5, (t5 + 1) * T5)
            for c8 in range(0, ND, 8):
                sl = slice(c8, c8 + 8)
                nc.sync.dma_start(xres[:, sl], hst_r[:, sl, tsl])
            return xres

        def w_slice(oc, d):
            if oc < QH:
                return wq_res[:, d, oc * P:(oc + 1) * P]
            if oc == QH:
                return wkv_res[:, d, 0:HD]
            return wkv_res[:, d, HD:2 * HD]

        cp_alt = [0]

        def rope(oc, proj, tsl):
            # rotate-half via two SBUF->SBUF DMA partition moves (the sign
            # lives in the host-precomputed signed-sin table), so RoPE costs
            # the PE nothing
            qraw = ropep.tile([P, T5], F32, tag="qraw", bufs=4, name="qraw")
            cp_alt[0] ^= 1
            if cp_alt[0]:
                nc.scalar.copy(qraw.bitcast(FP32R), proj[:])
            else:
                nc.vector.tensor_copy(qraw.bitcast(FP32R), proj[:])
            # SP queue: the ACT sequencer must stay clear for the exps —
            # a parked DMA dispatch head-of-line blocks everything behind it
            rotb = ropep.tile([P, T5], F32, tag="rotb", bufs=4, name="rotb")
            nc.sync.dma_start(rotb[0:P // 2].bitcast(FP32R),
                              qraw[P // 2:P].bitcast(FP32R))
            nc.sync.dma_start(rotb[P // 2:P].bitcast(FP32R),
                              qraw[0:P // 2].bitcast(FP32R))
            tmp = ropep.tile([P, T5], BF16, tag="tmp", bufs=3, name="tmp")
            nc.vector.tensor_mul(tmp[:], rotb[:], cs_res[:, 1, tsl])
            tmp2 = ropep.tile([P, T5], BF16, tag="tmp2", bufs=3, name="tmp2")
            nc.vector.tensor_mul(tmp2[:], qraw[:], cs_res[:, 0, tsl])
            dst = qT[:, oc, tsl] if oc < QH else kT[:, tsl]
            nc.vector.tensor_add(dst, tmp2[:], tmp[:])

        def make_ep(ocs, projs, t5):
            tsl = slice(t5 * T5, (t5 + 1) * T5)

            def ep(split_pe_tail=False):
                pe_tail = None
                for oc in ocs:
                    if oc == QH + 1:
                        vtmp = ropep.tile([P, T5], BF16, tag="vtmp", bufs=1,
                                          name="vtmp")
                        nc.scalar.copy(vtmp[:], projs[oc][:])

                        def v_tail(vtmp=vtmp):
                            v_ps = tp_psum.tile([P, T5], BF16, tag="tp",
                                                name="v_ps")
                            for i in range(4):
                                nc.tensor.transpose(
                                    v_ps[:, i * P:(i + 1) * P],
                                    vtmp[:, i * P:(i + 1) * P], ident)
                            nc.scalar.copy(vN[:, t5 * 4:(t5 + 1) * 4, :],
                                           v_ps[:])
                        if split_pe_tail:
                            pe_tail = v_tail
                        else:
                            v_tail()
                    else:
                        rope(oc, projs[oc], tsl)
                return pe_tail
            return ep

        # t5=0 startup schedule: interleave the Wkv / X / Wq(A-half) chunk
        # loads per-dJ so the matmul stream is never queued behind a bulk
        # transfer; the Wq B-half (Q2/Q3) and cos/sin load during pass B.
        MQH = 2 * P                              # A-half of the Wq columns
        xres0 = xpool.tile([P, ND, T5], BF16, tag="x", name="xres")
        nc.sync.dma_start(wkv_res[:, 0:4], wkv_r[:, 0:4])
        for dj in range(4):
            nc.sync.dma_start(xres0[:, dj], hst_r[:, dj, 0:T5])
        nc.sync.dma_start(wq_res[:, 0:4, 0:MQH], wq_r[:, 0:4, 0:MQH])

        pending_ep = None
        xres = xres0
        for t5 in range(NT5):
            for pi, (ocs, pool) in enumerate(((OCS_A, psA), (OCS_B, psB))):
                projs = {oc: pool.tile([P, T5], F32, tag=f"pj{pi}",
                                       name=f"pj{pi}_{oc}") for oc in ocs}
                for dJ in range(ND // 4):
                    if t5 == 0 and dJ > 0 and pi == 0:
                        dj4 = slice(dJ * 4, (dJ + 1) * 4)
                        nc.sync.dma_start(wkv_res[:, dj4], wkv_r[:, dj4])
                        nc.sync.dma_start(xres0[:, dj4], hst_r[:, dj4, 0:T5])
                        nc.sync.dma_start(wq_res[:, dj4, 0:MQH],
                                          wq_r[:, dj4, 0:MQH])
                    if t5 == 0 and pi == 1 and dJ < ND // 4 - 1:
                        # prefetch the next dJ's Wq B-half one step ahead
                        nj4 = slice((dJ + 1) * 4, (dJ + 2) * 4)
                        nc.sync.dma_start(wq_res[:, nj4, MQH:MQ],
                                          wq_r[:, nj4, MQH:MQ])
                        if dJ == 3:
                            xres_next = load_x(1)
                    for dj in range(4):
                        d = dJ * 4 + dj
                        for oc in ocs:
                            nc.tensor.matmul(projs[oc][:], w_slice(oc, d),
                                             xres[:, d],
                                             start=(d == 0), stop=(d == ND - 1))
                    if dJ == 0 and pending_ep is not None:
                        pending_ep()
                        pending_ep = None
                    if t5 == 0 and dJ == 0 and pi == 0:
                        nc.sync.dma_start(cc[:], cbf)
                    if t5 == 0 and dJ == ND // 4 - 1 and pi == 0:
                        # first Wq B-half lands during the pass-A tail so
                        # pass B's first matmuls don't wait; cos/sin chunk
                        # must be EMITTED before the epilogue that reads it
                        # (else Tile orders the load after the read)
                        nc.sync.dma_start(wq_res[:, 0:4, MQH:MQ],
                                          wq_r[:, 0:4, MQH:MQ])
                    if dJ == ND // 4 - 1 and pi == 0:
                        csl = slice(t5 * T5, (t5 + 1) * T5)
                        nc.sync.dma_start(cs_res[:, :, csl].bitcast(FP32R),
                                          cs_d[:, :, csl].bitcast(FP32R))
                if pi == 0 and 0 < t5 < NT5 - 1:
                    xres_next = load_x(t5 + 1)
                pending_ep = make_ep(ocs, projs, t5)
            xres = xres_next
        # final epilogue: emit the copies/rope chains now (they read phase-1
        # PSUM), but defer the V-transposes into phase 2 so the PE goes
        # straight from the last projection into the first scores
        final_pe_tail = pending_ep(split_pe_tail=True)
        pending_ep = None

    # ---- phases 2+3: cross-head pipelined attention + interleaved out-proj
    # Head i's softmax finalize (ones-matmul -> recip -> normalize) is
    # deferred into head i+1's prologue so its DVE tail hides under PE
    # work.  The out-projection of head-group g-1 is interleaved one
    # (tcn, ec) chunk per kc-slot of group g's heads: the 4-matmul chunk
    # keeps the PE busy while ACT computes the exps (612ns/chunk vs the
    # 426ns score+PV pair).
    wo_r = wo.rearrange("(ho p) e -> p ho e", p=P)  # [128, 4, 4096]
    LA = 2                                          # score lookahead depth
    NEC = D // T5                                   # 8 out-proj col chunks
    # PSUM stack order mirrors phase 1's drain order: st lands on psA's
    # banks (drained first), acc/out over psA/psB.  The acc ring (3) hosts
    # both oacc and lacc: per head the two allocations advance the ring so
    # head i+1's oacc lands on head i's freed lacc slot.
    with tc.tile_pool(name="wopool", bufs=1) as wopool, \
         tc.tile_pool(name="attn", bufs=2) as apool, \
         tc.tile_pool(name="p_pool", bufs=6) as ppool, \
         tc.tile_pool(name="obuf", bufs=4) as obuf, \
         tc.tile_pool(name="st_ps", bufs=2, space="PSUM") as st_psum, \
         tc.tile_pool(name="acc_ps", bufs=3, space="PSUM") as acc_psum, \
         tc.tile_pool(name="out_ps", bufs=2, space="PSUM") as out_psum:
        wo_sb = wopool.tile([P, QH, D], BF16)       # resident Wo shard (4 MB)
        for ec in range(NEC):
            esl = slice(ec * T5, (ec + 1) * T5)
            nc.sync.dma_start(wo_sb[:, :, esl], wo_r[:, :, esl])

        groups = [(b, qh) for b in range(B) for qh in range(2)]
        state = {"fin": None, "ob": None}

        def outproj_chunk(g, c, last_group=False):
            """Emit out-proj chunk c (of 32) for group g: 4 accumulating
            matmuls into one PSUM bank + drain copy (+ store DMA)."""
            q0 = groups[g][0] * S + groups[g][1] * T5
            tcn = q0 // P + c // NEC
            ec = c % NEC
            if ec == 0:
                state["ob"] = [obuf.tile([P, D // 2], BF16, tag="ob", bufs=3,
                                         name="ob") for _ in range(2)]
            ob = state["ob"][ec // 4]
            esl = slice(ec * T5, (ec + 1) * T5)
            out_ps = out_psum.tile([P, T5], F32, tag="outp", name="out_ps")
            for hc in range(QH):
                nc.tensor.matmul(out_ps[:],
                                 oT[:, hc, tcn * P:(tcn + 1) * P],
                                 wo_sb[:, hc, esl],
                                 start=(hc == 0), stop=(hc == QH - 1))
            osl = slice((ec % 4) * T5, (ec % 4 + 1) * T5)
            if ec % 2 == 0:
                nc.vector.tensor_copy(ob[:, osl], out_ps[:])
            else:
                nc.scalar.copy(ob[:, osl], out_ps[:])
            if last_group:
                # fine-grained stores so the final drain isn't one big DMA
                if c >= 3 * NEC:          # final token chunk: store per-ec
                    nc.sync.dma_start(
                        out_ap[tcn * P:(tcn + 1) * P, esl],
                        ob[:, (ec % 4) * T5:(ec % 4 + 1) * T5])
                elif ec % 2 == 1:
                    psl = slice((ec - 1) * T5, (ec + 1) * T5)
                    hsl = slice((ec % 4 - 1) * T5, (ec % 4 + 1) * T5)
                    nc.sync.dma_start(
                        out_ap[tcn * P:(tcn + 1) * P, psl], ob[:, hsl])
            elif ec % 4 == 3:
                half_i = ec // 4
                nc.sync.dma_start(
                    out_ap[tcn * P:(tcn + 1) * P,
                           half_i * (D // 2):(half_i + 1) * (D // 2)],
                    ob[:])

        def head_body(i):
            g, h = i // 4, i % 4
            b, qh = groups[g]
            q0 = b * S + qh * T5
            qsl = slice(q0, q0 + T5)
            # fixed 2-alloc-per-head ring order (oacc then lacc) so head
            # i+1's oacc always lands on head i-1's freed slot, never on a
            # slot whose release depends on this head's own finalize
            oacc = acc_psum.tile([P, T5], F32, tag="acc", name="oacc")
            lacc = acc_psum.tile([P, T5], F32, tag="acc", name="lacc")

            def score(kc):
                ksl = slice(b * S + kc * P, b * S + (kc + 1) * P)
                st = st_psum.tile([P, T5], F32, tag="st", name="st")
                nc.tensor.matmul(st[:], kT[:, ksl], qT[:, h, qsl],
                                 start=True, stop=True)
                p_sb = ppool.tile([P, T5], BF16, tag="p", bufs=8, name="p_sb")
                nc.scalar.activation(p_sb[:], st[:], Exp, scale=SCALE)
                return p_sb

            ps = [score(kc) for kc in range(LA)]
            if i == 0:
                final_pe_tail()     # deferred phase-1 V-transposes
            pairs, quads = [], []   # l add-tree: pairs -> quads -> one
            for kc in range(NKC):
                if kc + LA < NKC:
                    ps.append(score(kc + LA))
                p_sb = ps[kc]
                nc.tensor.matmul(oacc[:], vN[:, b * NKC + kc, :], p_sb[:],
                                 start=(kc == 0), stop=(kc == NKC - 1))
                # flush the previous head's finalize here (max PE cover for
                # its DVE tail); for h==0 it must land before chunk kc=0,
                # which reads the oT row that finalize writes
                if kc == 0 and h == 0 and state["fin"] is not None:
                    state["fin"]()
                    state["fin"] = None
                if g > 0:
                    # one out-proj chunk of the previous group per kc-slot
                    outproj_chunk(g - 1, h * NKC + kc)
                else:
                    # group 0 has no out-proj to host and is exp-paced on
                    # ACT: spend the idle PE slots summing the softmax
                    # denominator by per-chunk ones-matmuls (no DVE tree)
                    nc.tensor.matmul(lacc[:], ones, p_sb[:],
                                     start=(kc == 0), stop=(kc == NKC - 1))
                if kc == 0 and state["fin"] is not None:
                    state["fin"]()
                    state["fin"] = None
                if g > 0 and kc % 2 == 1:
                    s = ppool.tile([P, T5], BF16, tag="ls", bufs=4, name="ls")
                    nc.vector.tensor_add(s[:], ps[kc - 1][:], p_sb[:])
                    pairs.append(s)
                    if len(pairs) == 2:
                        q = ppool.tile([P, T5], BF16, tag="lq", bufs=3,
                                       name="lq")
                        nc.vector.tensor_add(q[:], pairs[0][:], pairs[1][:])
                        pairs = []
                        quads.append(q)
            if g > 0:
                sfin = ppool.tile([P, T5], BF16, tag="lf", bufs=2, name="lf")
                nc.vector.tensor_add(sfin[:], quads[0][:], quads[1][:])
            else:
                sfin = None

            def fin():
                if sfin is not None:
                    nc.tensor.matmul(lacc[:], ones, sfin[:],
                                     start=True, stop=True)
                recip = apool.tile([P, T5], F32, tag="recip", name="recip")
                nc.vector.reciprocal(recip[:], lacc[:])
                nc.vector.tensor_mul(oT[:, h, qsl], oacc[:], recip[:])
            state["fin"] = fin

        for i in range(4 * len(groups)):
            head_body(i)
        state["fin"]()
        state["fin"] = None
        for c in range(4 * NEC):
            outproj_chunk(len(groups) - 1, c, last_group=True)


def _get_nc(nbody=1):
    key = ("nc", nbody)
    if key in _CACHE:
        return _CACHE[key]
    import concourse.tile as tile
    from concourse import bacc, mybir

    F32 = mybir.dt.float32
    BF16 = mybir.dt.bfloat16
    nc = bacc.Bacc("TRN2", target_bir_lowering=False, debug=False)
    hst = nc.dram_tensor("hst", [D, TT], BF16, kind="ExternalInput").ap()
    cs = nc.dram_tensor("cs", [HD, 2 * TT], F32, kind="ExternalInput").ap()
    wq = nc.dram_tensor("wq", [D, MQ], BF16, kind="ExternalInput").ap()
    wkv = nc.dram_tensor("wkv", [D, 2 * HD], BF16, kind="ExternalInput").ap()
    wo = nc.dram_tensor("wo", [MQ, D], BF16, kind="ExternalInput").ap()
    cbf = nc.dram_tensor("cbf", [P, 2 * P], BF16, kind="ExternalInput").ap()
    out = nc.dram_tensor("out", [TT, D], BF16, kind="ExternalOutput").ap()
    with tile.TileContext(nc) as tc:
        for _ in range(nbody):
            with ExitStack() as ctx:
                tc.ctx = ctx
                _build_kernel(tc, out, (hst,
                                        cs.rearrange('p (a t) -> p a t', a=2),
                                        wq, wkv, wo,
                                        cbf.rearrange('p (t q) -> p t q', t=2)))
    nc.compile()
    _CACHE[key] = nc
    return nc


def _in_maps(hidden_states, cos_table, sin_table, Wq, Wk, Wv, Wo):
    import ml_dtypes
    BF = ml_dtypes.bfloat16
    hst = np.ascontiguousarray(np.asarray(hidden_states, dtype=np.float32)
                               .reshape(TT, D).T).astype(BF)
    cost = np.asarray(cos_table, dtype=np.float32).reshape(TT, HD).T
    sint = np.asarray(sin_table, dtype=np.float32).reshape(TT, HD).T.copy()
    sint[0:HD // 2] *= -1.0     # sign of rotate-half folded into the table
    cs = np.ascontiguousarray(np.stack([cost, sint], axis=1)
                              .reshape(HD, 2 * TT))
    Wq = np.asarray(Wq, dtype=np.float32).astype(BF)
    Wk = np.asarray(Wk, dtype=np.float32).astype(BF)
    Wv = np.asarray(Wv, dtype=np.float32).astype(BF)
    Wo = np.asarray(Wo, dtype=np.float32).astype(BF)
    ident = np.eye(P, dtype=np.float32)
    ones = np.ones((P, P), dtype=np.float32)
    cbf = np.concatenate([ident, ones], axis=1).astype(BF)
    maps = []
    for c in range(NCORES):
        wkv = np.stack([Wk[:, c * HD:(c + 1) * HD],
                        Wv[:, c * HD:(c + 1) * HD]], axis=1)
        maps.append({
            "hst": hst,
            "cs": cs,
            "wq": np.ascontiguousarray(Wq[:, c * MQ:(c + 1) * MQ]),
            "wkv": np.ascontiguousarray(wkv.reshape(D, 2 * HD)),
            "wo": np.ascontiguousarray(Wo[c * MQ:(c + 1) * MQ, :]),
            "cbf": cbf,
        })
    return maps


# inputs identical on every core: sent once and broadcast by shard_map
_REPLICATED = {"hst", "cs", "cbf"}


def _get_runner(nbody=1):
    """Build the 8-core SPMD executable once (mirrors the multi-core branch
    of bass2jax.run_bass_via_pjrt, but cached so repeat calls don't re-jit
    or re-compile the NEFF).  Replicated inputs ship once; the zero output
    buffers the NEFF writes into are created on-device."""
    key = ("runner", nbody)
    if key in _CACHE:
        return _CACHE[key]
    import jax
    from jax.sharding import Mesh, PartitionSpec
    from jax.experimental.shard_map import shard_map
    import concourse.mybir as mybir
    from concourse import bass2jax

    nc = _get_nc(nbody)
    bass2jax.install_neuronx_cc_hook()

    part_name = nc.partition_id_tensor.name if nc.partition_id_tensor else None
    in_names, out_names, out_avals, zero_outs = [], [], [], []
    for alloc in nc.m.functions[0].allocations:
        if not isinstance(alloc, mybir.MemoryLocationSet):
            continue
        name = alloc.memorylocations[0].name
        if alloc.kind == "ExternalInput":
            if name != part_name:
                in_names.append(name)
        elif alloc.kind == "ExternalOutput":
            out_names.append(name)
            shape = tuple(alloc.tensor_shape)
            dtype = mybir.dt.np(alloc.dtype)
            out_avals.append(jax.core.ShapedArray(shape, dtype))
            zero_outs.append(np.zeros(shape, dtype))
    n_params = len(in_names)
    all_names = in_names + out_names
    if part_name is not None:
        all_names = all_names + [part_name]

    def _body(*args):
        operands = list(args)
        if part_name is not None:
            operands.append(bass2jax.partition_id_tensor())
        outs = bass2jax._bass_exec_p.bind(
            *operands,
            out_avals=tuple(out_avals),
            in_names=tuple(all_names),
            out_names=tuple(out_names),
            lowering_input_output_aliases=(),
            sim_require_finite=True,
            sim_require_nnan=True,
            nc=nc,
        )
        return tuple(outs)

    devices = jax.devices()[:NCORES]
    assert len(devices) == NCORES, (
        f"need {NCORES} NeuronCores, jax.devices() shows {len(jax.devices())}")
    mesh = Mesh(np.asarray(devices), ("core",))
    in_specs = tuple(PartitionSpec() if n in _REPLICATED
                     else PartitionSpec("core") for n in in_names) \
        + (PartitionSpec("core"),) * len(out_names)
    sharded = jax.jit(
        shard_map(_body, mesh=mesh,
                  in_specs=in_specs,
                  out_specs=(PartitionSpec("core"),) * len(out_names),
                  check_rep=False),
        keep_unused=True,
    )
    runner = (sharded, mesh, in_names, out_names, out_avals, zero_outs)
    _CACHE[key] = runner
    return runner


def _concat_inputs(maps):
    sharded, mesh, in_names, out_names, out_avals, zero_outs = _get_runner()
    concat_in = [maps[0][n] if n in _REPLICATED
                 else np.concatenate([maps[c][n] for c in range(NCORES)], axis=0)
                 for n in in_names]
    concat_zeros = [np.zeros((NCORES * z.shape[0], *z.shape[1:]), z.dtype)
                    for z in zero_outs]
    return concat_in + concat_zeros


def _run(maps):
    sharded, mesh, in_names, out_names, out_avals, zero_outs = _get_runner()
    out_arrs = sharded(*_concat_inputs(maps))
    return [np.asarray(out_arrs[0]).reshape(NCORES, *out_avals[0].shape)[c]
            for c in range(NCORES)]


def kernel(hidden_states, cos_table, sin_table, Wq, Wk, Wv, Wo):
    maps = _in_maps(hidden_states, cos_table, sin_table, Wq, Wk, Wv, Wo)
    parts = np.stack(_run(maps))
    out = parts.astype(np.float32).sum(axis=0)
    return out.reshape(B, S, D)


# revision 70
# speedup vs baseline: 1.0564x; 1.0564x over previous
"""Fused GQA attention block (QKV proj + RoPE + SDPA + out proj) on 8 TRN2
NeuronCores.

Sharding: tensor-parallel over heads. Core c owns kv-head c (q-heads
4c..4c+3): Wq/Wk/Wv column shards, Wo row shard. Each core computes a
full-shape partial of the output projection; the host sums the 8 partials.

All matmul operands are bf16 (PSUM accumulation stays fp32; measured rel
err ~6e-3 vs the 2e-2 gate): halves DMA traffic and SBUF footprint at
unchanged PE speed (1 cycle/row).

Per-core dataflow:
  phase 1: Q^T/K^T/V^T = W^T X^T accumulated over D, two passes of 3
           output chunks (3+3 PSUM banks); X tile and all weights
           SBUF-resident (X chunk-streamed one token-tile ahead).  Each
           pass's epilogue (RoPE / V-transpose) is deferred into the next
           pass's matmul stream so the PE never waits on PSUM drains:
           RoPE's rotate-half runs as two SBUF->SBUF DMA partition moves
           (sign folded into the host-built signed-sin table) + two muls
           and an add on DVE; V^T re-transposed to natural [token, hd]
           chunks on the PE.  The final epilogue's V-transposes defer
           into phase 2 under the first scores.
  phase 2: per (batch, q-head): S^T = K^T.T @ Q^T; P^T = exp(S^T*scale)
           on ACT straight out of PSUM (bf16 out); O^T = V.T @ P^T.
           Scores issue LA k-chunks ahead of the P@V accumulation; each
           head's finalize (denominator matmul -> reciprocal ->
           normalize) defers into the next head's prologue so its DVE
           tail hides under PE work.  Denominators: group 0 sums exp
           chunks by per-chunk ones-matmuls (free PE slots - it hosts no
           out-proj and is exp-paced); later groups use a 7-add bf16 DVE
           tree + one 512-row ones-matmul.  No row-max subtraction:
           scores are O(5), exp cannot overflow.
  phase 3: out_partial = O^T.T @ Wo shard (Wo SBUF-resident), one
           (token-chunk, 512-col) chunk interleaved into each phase-2
           kc-slot (the 4-matmul chunk keeps the PE busy while ACT runs
           the exps); stores stream to DRAM in bf16, fine-grained for the
           trailing group so the final drain is short.
"""

from contextlib import ExitStack

import numpy as np

B, S, D = 2, 1024, 4096
HQ, HKV, HD = 32, 8, 128
NCORES = 8
QH = HQ // NCORES          # 4 q heads per core
MQ = QH * HD               # 512 q-projection columns per core
TT = B * S                 # 2048 tokens
P = 128
T5 = 512                   # token macro-tile
NT5 = TT // T5             # 4
ND = D // P                # 32 contraction chunks
NKC = S // P               # 8 key chunks per batch
SCALE = HD ** -0.5

_CACHE = {}


def _build_kernel(tc, out_ap, ins):
    from concourse import mybir

    nc = tc.nc
    F32 = mybir.dt.float32
    BF16 = mybir.dt.bfloat16
    FP32R = mybir.dt.float32r
    Exp = mybir.ActivationFunctionType.Exp

    hst, cs_d, wq, wkv, wo, cbf = ins

    ctx = tc.ctx  # set by caller
    const = ctx.enter_context(tc.tile_pool(name="const", bufs=1))
    persist = ctx.enter_context(tc.tile_pool(name="persist", bufs=1))
    # ropep/tp stay open for the whole kernel: if they closed with phase 1,
    # phase 2's pools would allocate over their released zones and wait for
    # the final RoPE epilogue's DMA chains before the first exp; keeping tp
    # open also lets the final V-transposes defer into phase 2
    ropep = ctx.enter_context(tc.tile_pool(name="ropep", bufs=2))
    tp_psum = ctx.enter_context(tc.tile_pool(name="tp_ps", bufs=1,
                                             space="PSUM"))

    # ---- constants: identity+ones (bf16) --------------------------------
    # (tile allocated here; its DMA is emitted after the startup-critical
    # X/weight chunks so the first matmuls aren't queued behind it)
    cc = const.tile([P, 2, P], BF16)
    ident = cc[:, 0]
    ones = cc[:, 1]
    # ---- persistent activations (all bf16) ------------------------------
    qT = persist.tile([P, QH, TT], BF16)       # Q^T per head
    kT = persist.tile([P, TT], BF16)           # K^T (one kv head)
    vN = persist.tile([P, TT // P, P], BF16)   # V natural [tok, hd] chunks
    oT = persist.tile([P, QH, TT], BF16)       # attention out, transposed

    # ---- phases 0+1: projections + RoPE ---------------------------------
    # Two passes of 3 output chunks per token tile (3+3 PSUM banks), with
    # each pass's RoPE/transpose epilogue deferred into the next pass's
    # matmul stream so the PE never waits on PSUM drains or DVE chains.
    wq_r = wq.rearrange("(o p) m -> p o m", p=P)    # [128, 32, 512]
    wkv_r = wkv.rearrange("(o p) m -> p o m", p=P)  # [128, 32, 256]
    hst_r = hst.rearrange("(o p) t -> p o t", p=P)  # [128, 32, 2048]
    OCS_A, OCS_B = (QH, 0, 1), (QH + 1, 2, 3)       # K,Q0,Q1 | V,Q2,Q3
    # (V leads pass B's epilogue so its PSUM bank + the transpose bank drain
    # early; phase 2's pools then land on banks that free first.)

    with tc.tile_pool(name="ph1", bufs=1) as ph1, \
         tc.tile_pool(name="xpool", bufs=2) as xpool, \
         tc.tile_pool(name="psA", bufs=3, space="PSUM") as psA, \
         tc.tile_pool(name="psB", bufs=3, space="PSUM") as psB:
        wq_res = ph1.tile([P, ND, MQ], BF16)    # resident Wq shard (4 MB)
        wkv_res = ph1.tile([P, ND, 2 * HD], BF16)
        # cos/sin lives in the ctx-scoped pool: the final epilogue's rope
        # muls read it after phase 1's pools close; if it sat in ph1, the
        # ph1 zone release (and with it phase 2's pool opens, exps, and
        # first l-matmul) would wait for those muls
        cs_res = ropep.tile([P, 2, TT], F32, tag="cs", bufs=1, name="cs_res")

        def load_x(t5):
            # 4 chunked DMAs: pass A's first matmuls wait only on chunk 0,
            # and other queue traffic can interleave between chunks
            xres = xpool.tile([P, ND, T5], BF16, tag="x", name="xres")
            tsl = slice(t5 * T5, (t5 + 1) * T5)
            for c8 in range(0, ND, 8):
                sl = slice(c8, c8 + 8)
                nc.sync.dma_start(xres[:, sl], hst_r[:, sl, tsl])
            return xres

        def w_slice(oc, d):
            if oc < QH:
                return wq_res[:, d, oc * P:(oc + 1) * P]
            if oc == QH:
                return wkv_res[:, d, 0:HD]
            return wkv_res[:, d, HD:2 * HD]

        cp_alt = [0]

        def rope(oc, proj, tsl):
            # rotate-half via two SBUF->SBUF DMA partition moves (the sign
            # lives in the host-precomputed signed-sin table), so RoPE costs
            # the PE nothing
            qraw = ropep.tile([P, T5], F32, tag="qraw", bufs=4, name="qraw")
            cp_alt[0] ^= 1
            if cp_alt[0]:
                nc.scalar.copy(qraw.bitcast(FP32R), proj[:])
            else:
                nc.vector.tensor_copy(qraw.bitcast(FP32R), proj[:])
            # SP queue: the ACT sequencer must stay clear for the exps —
            # a parked DMA dispatch head-of-line blocks everything behind it
            rotb = ropep.tile([P, T5], F32, tag="rotb", bufs=4, name="rotb")
            nc.sync.dma_start(rotb[0:P // 2].bitcast(FP32R),
                              qraw[P // 2:P].bitcast(FP32R))
            nc.sync.dma_start(rotb[P // 2:P].bitcast(FP32R),
                              qraw[0:P // 2].bitcast(FP32R))
            tmp = ropep.tile([P, T5], BF16, tag="tmp", bufs=3, name="tmp")
            nc.vector.tensor_mul(tmp[:], rotb[:], cs_res[:, 1, tsl])
            tmp2 = ropep.tile([P, T5], BF16, tag="tmp2", bufs=3, name="tmp2")
            nc.vector.tensor_mul(tmp2[:], qraw[:], cs_res[:, 0, tsl])
            dst = qT[:, oc, tsl] if oc < QH else kT[:, tsl]
            nc.vector.tensor_add(dst, tmp2[:], tmp[:])

        def make_ep(ocs, projs, t5):
            tsl = slice(t5 * T5, (t5 + 1) * T5)

            def ep(split_pe_tail=False):
                pe_tail = None
                for oc in ocs:
                    if oc == QH + 1:
                        vtmp = ropep.tile([P, T5], BF16, tag="vtmp", bufs=1,
                                          name="vtmp")
                        nc.scalar.copy(vtmp[:], projs[oc][:])

                        def v_tail(vtmp=vtmp):
                            v_ps = tp_psum.tile([P, T5], BF16, tag="tp",
                                                name="v_ps")
                            for i in range(4):
                                nc.tensor.transpose(
                                    v_ps[:, i * P:(i + 1) * P],
                                    vtmp[:, i * P:(i + 1) * P], ident)
                            nc.scalar.copy(vN[:, t5 * 4:(t5 + 1) * 4, :],
                                           v_ps[:])
                        if split_pe_tail:
                            pe_tail = v_tail
                        else:
                            v_tail()
                    else:
                        rope(oc, projs[oc], tsl)
                return pe_tail
            return ep

        # t5=0 startup schedule: interleave the Wkv / X / Wq(A-half) chunk
        # loads per-dJ so the matmul stream is never queued behind a bulk
        # transfer; the Wq B-half (Q2/Q3) and cos/sin load during pass B.
        MQH = 2 * P                              # A-half of the Wq columns
        xres0 = xpool.tile([P, ND, T5], BF16, tag="x", name="xres")
        nc.sync.dma_start(wkv_res[:, 0:4], wkv_r[:, 0:4])
        for dj in range(4):
            nc.sync.dma_start(xres0[:, dj], hst_r[:, dj, 0:T5])
        nc.sync.dma_start(wq_res[:, 0:4, 0:MQH], wq_r[:, 0:4, 0:MQH])

        pending_ep = None
        xres = xres0
        for t5 in range(NT5):
            for pi, (ocs, pool) in enumerate(((OCS_A, psA), (OCS_B, psB))):
                projs = {oc: pool.tile([P, T5], F32, tag=f"pj{pi}",
                                       name=f"pj{pi}_{oc}") for oc in ocs}
                for dJ in range(ND // 4):
                    if t5 == 0 and dJ > 0 and pi == 0:
                        dj4 = slice(dJ * 4, (dJ + 1) * 4)
                        nc.sync.dma_start(wkv_res[:, dj4], wkv_r[:, dj4])
                        nc.sync.dma_start(xres0[:, dj4], hst_r[:, dj4, 0:T5])
                        nc.sync.dma_start(wq_res[:, dj4, 0:MQH],
                                          wq_r[:, dj4, 0:MQH])
                    if t5 == 0 and pi == 1 and dJ < ND // 4 - 1:
                        # prefetch the next dJ's Wq B-half one step ahead
                        nj4 = slice((dJ + 1) * 4, (dJ + 2) * 4)
                        nc.sync.dma_start(wq_res[:, nj4, MQH:MQ],
                                          wq_r[:, nj4, MQH:MQ])
                        if dJ == 3:
                            xres_next = load_x(1)
                    for dj in range(4):
                        d = dJ * 4 + dj
                        for oc in ocs:
                            nc.tensor.matmul(projs[oc][:], w_slice(oc, d),
                                             xres[:, d],
                                             start=(d == 0), stop=(d == ND - 1))
                    if dJ == 0 and pending_ep is not None:
                        pending_ep()
                        pending_ep = None
                    if t5 == 0 and dJ == 0 and pi == 0:
                        nc.sync.dma_start(cc[:], cbf)
                    if t5 == 0 and dJ == ND // 4 - 1 and pi == 0:
                        # first Wq B-half lands during the pass-A tail so
                        # pass B's first matmuls don't wait; cos/sin chunk
                        # must be EMITTED before the epilogue that reads it
                        # (else Tile orders the load after the read)
                        nc.sync.dma_start(wq_res[:, 0:4, MQH:MQ],
                                          wq_r[:, 0:4, MQH:MQ])
                    if dJ == ND // 4 - 1 and pi == 0:
                        csl = slice(t5 * T5, (t5 + 1) * T5)
                        nc.sync.dma_start(cs_res[:, :, csl].bitcast(FP32R),
                                          cs_d[:, :, csl].bitcast(FP32R))
                if pi == 0 and 0 < t5 < NT5 - 1:
                    xres_next = load_x(t5 + 1)
                pending_ep = make_ep(ocs, projs, t5)
            xres = xres_next
        # final epilogue: emit the copies/rope chains now (they read phase-1
        # PSUM), but defer the V-transposes into phase 2 so the PE goes
        # straight from the last projection into the first scores
        final_pe_tail = pending_ep(split_pe_tail=True)
        pending_ep = None

    # ---- phases 2+3: cross-head pipelined attention + interleaved out-proj
    # Head i's softmax finalize (ones-matmul -> recip -> normalize) is
    # deferred into head i+1's prologue so its DVE tail hides under PE
    # work.  The out-projection of head-group g-1 is interleaved one
    # (tcn, ec) chunk per kc-slot of group g's heads: the 4-matmul chunk
    # keeps the PE busy while ACT computes the exps (612ns/chunk vs the
    # 426ns score+PV pair).
    wo_r = wo.rearrange("(ho p) e -> p ho e", p=P)  # [128, 4, 4096]
    LA = 2                                          # score lookahead depth
    NEC = D // T5                                   # 8 out-proj col chunks
    # PSUM stack order mirrors phase 1's drain order: st lands on psA's
    # banks (drained first), acc/out over psA/psB.  The acc ring (3) hosts
    # both oacc and lacc: per head the two allocations advance the ring so
    # head i+1's oacc lands on head i's freed lacc slot.
    with tc.tile_pool(name="wopool", bufs=1) as wopool, \
         tc.tile_pool(name="attn", bufs=2) as apool, \
         tc.tile_pool(name="p_pool", bufs=6) as ppool, \
         tc.tile_pool(name="obuf", bufs=4) as obuf, \
         tc.tile_pool(name="st_ps", bufs=2, space="PSUM") as st_psum, \
         tc.tile_pool(name="acc_ps", bufs=3, space="PSUM") as acc_psum, \
         tc.tile_pool(name="out_ps", bufs=2, space="PSUM") as out_psum:
        wo_sb = wopool.tile([P, QH, D], BF16)       # resident Wo shard (4 MB)
        for ec in range(NEC):
            esl = slice(ec * T5, (ec + 1) * T5)
            nc.sync.dma_start(wo_sb[:, :, esl], wo_r[:, :, esl])

        groups = [(b, qh) for b in range(B) for qh in range(2)]
        state = {"fin": None, "ob": None}

        def outproj_chunk(g, c, last_group=False):
            """Emit out-proj chunk c (of 32) for group g: 4 accumulating
            matmuls into one PSUM bank + drain copy (+ store DMA)."""
            q0 = groups[g][0] * S + groups[g][1] * T5
            tcn = q0 // P + c // NEC
            ec = c % NEC
            if ec == 0:
                state["ob"] = [obuf.tile([P, D // 2], BF16, tag="ob", bufs=3,
                                         name="ob") for _ in range(2)]
            ob = state["ob"][ec // 4]
            esl = slice(ec * T5, (ec + 1) * T5)
            out_ps = out_psum.tile([P, T5], F32, tag="outp", name="out_ps")
            for hc in range(QH):
                nc.tensor.matmul(out_ps[:],
                                 oT[:, hc, tcn * P:(tcn + 1) * P],
                                 wo_sb[:, hc, esl],
                                 start=(hc == 0), stop=(hc == QH - 1))
            osl = slice((ec % 4) * T5, (ec % 4 + 1) * T5)
            if ec % 2 == 0:
                nc.vector.tensor_copy(ob[:, osl], out_ps[:])
            else:
                nc.scalar.copy(ob[:, osl], out_ps[:])
            if last_group:
                # fine-grained stores so the final drain isn't one big DMA
                if c >= 3 * NEC:          # final token chunk: store per-ec
                    nc.sync.dma_start(
                        out_ap[tcn * P:(tcn + 1) * P, esl],
                        ob[:, (ec % 4) * T5:(ec % 4 + 1) * T5])
                elif ec % 2 == 1:
                    psl = slice((ec - 1) * T5, (ec + 1) * T5)
                    hsl = slice((ec % 4 - 1) * T5, (ec % 4 + 1) * T5)
                    nc.sync.dma_start(
                        out_ap[tcn * P:(tcn + 1) * P, psl], ob[:, hsl])
            elif ec % 4 == 3:
                half_i = ec // 4
                nc.sync.dma_start(
                    out_ap[tcn * P:(tcn + 1) * P,
                           half_i * (D // 2):(half_i + 1) * (D // 2)],
                    ob[:])

        def head_body(i):
            g, h = i // 4, i % 4
            b, qh = groups[g]
            q0 = b * S + qh * T5
            qsl = slice(q0, q0 + T5)
            # fixed 2-alloc-per-head ring order (oacc then lacc) so head
            # i+1's oacc always lands on head i-1's freed slot, never on a
            # slot whose release depends on this head's own finalize
            oacc = acc_psum.tile([P, T5], F32, tag="acc", name="oacc")
            lacc = acc_psum.tile([P, T5], F32, tag="acc", name="lacc")

            def score(kc):
                ksl = slice(b * S + kc * P, b * S + (kc + 1) * P)
                st = st_psum.tile([P, T5], F32, tag="st", name="st")
                nc.tensor.matmul(st[:], kT[:, ksl], qT[:, h, qsl],
                                 start=True, stop=True)
                p_sb = ppool.tile([P, T5], BF16, tag="p", bufs=8, name="p_sb")
                nc.scalar.activation(p_sb[:], st[:], Exp, scale=SCALE)
                return p_sb

            ps = [score(kc) for kc in range(LA)]
            if i == 0:
                final_pe_tail()     # deferred phase-1 V-transposes
            pairs, quads = [], []   # l add-tree: pairs -> quads -> one
            for kc in range(NKC):
                if kc + LA < NKC:
                    ps.append(score(kc + LA))
                p_sb = ps[kc]
                nc.tensor.matmul(oacc[:], vN[:, b * NKC + kc, :], p_sb[:],
                                 start=(kc == 0), stop=(kc == NKC - 1))
                # flush the previous head's finalize here (max PE cover for
                # its DVE tail); for h==0 it must land before chunk kc=0,
                # which reads the oT row that finalize writes
                if kc == 0 and h == 0 and state["fin"] is not None:
                    state["fin"]()
                    state["fin"] = None
                if g > 0:
                    # one out-proj chunk of the previous group per kc-slot
                    outproj_chunk(g - 1, h * NKC + kc)
                else:
                    # group 0 has no out-proj to host and is exp-paced on
                    # ACT: spend the idle PE slots summing the softmax
                    # denominator by per-chunk ones-matmuls (no DVE tree)
                    nc.tensor.matmul(lacc[:], ones, p_sb[:],
                                     start=(kc == 0), stop=(kc == NKC - 1))
                if kc == 0 and state["fin"] is not None:
                    state["fin"]()
                    state["fin"] = None
                if g > 0 and kc % 2 == 1:
                    s = ppool.tile([P, T5], BF16, tag="ls", bufs=4, name="ls")
                    nc.vector.tensor_add(s[:], ps[kc - 1][:], p_sb[:])
                    pairs.append(s)
                    if len(pairs) == 2:
                        q = ppool.tile([P, T5], BF16, tag="lq", bufs=3,
                                       name="lq")
                        nc.vector.tensor_add(q[:], pairs[0][:], pairs[1][:])
                        pairs = []
                        quads.append(q)
            if g > 0:
                sfin = ppool.tile([P, T5], BF16, tag="lf", bufs=2, name="lf")
                nc.vector.tensor_add(sfin[:], quads[0][:], quads[1][:])
            else:
                sfin = None

            def fin():
                if sfin is not None:
                    nc.tensor.matmul(lacc[:], ones, sfin[:],
                                     start=True, stop=True)
                recip = apool.tile([P, T5], F32, tag="recip", name="recip")
                nc.vector.reciprocal(recip[:], lacc[:])
                nc.vector.tensor_mul(oT[:, h, qsl], oacc[:], recip[:])
            state["fin"] = fin

        for i in range(4 * len(groups)):
            head_body(i)
        state["fin"]()
        state["fin"] = None
        for c in range(4 * NEC):
            outproj_chunk(len(groups) - 1, c, last_group=True)


def _get_nc(nbody=1):
    key = ("nc", nbody)
    if key in _CACHE:
        return _CACHE[key]
    import concourse.tile as tile
    from concourse import bacc, mybir

    F32 = mybir.dt.float32
    BF16 = mybir.dt.bfloat16
    nc = bacc.Bacc("TRN2", target_bir_lowering=False, debug=False)
    hst = nc.dram_tensor("hst", [D, TT], BF16, kind="ExternalInput").ap()
    cs = nc.dram_tensor("cs", [HD, 2 * TT], F32, kind="ExternalInput").ap()
    wq = nc.dram_tensor("wq", [D, MQ], BF16, kind="ExternalInput").ap()
    wkv = nc.dram_tensor("wkv", [D, 2 * HD], BF16, kind="ExternalInput").ap()
    wo = nc.dram_tensor("wo", [MQ, D], BF16, kind="ExternalInput").ap()
    cbf = nc.dram_tensor("cbf", [P, 2 * P], BF16, kind="ExternalInput").ap()
    out = nc.dram_tensor("out", [TT, D], BF16, kind="ExternalOutput").ap()
    with tile.TileContext(nc) as tc:
        for _ in range(nbody):
            with ExitStack() as ctx:
                tc.ctx = ctx
                _build_kernel(tc, out, (hst,
                                        cs.rearrange('p (a t) -> p a t', a=2),
                                        wq, wkv, wo,
                                        cbf.rearrange('p (t q) -> p t q', t=2)))
    nc.compile()
    _CACHE[key] = nc
    return nc


def _in_maps(hidden_states, cos_table, sin_table, Wq, Wk, Wv, Wo):
    import ml_dtypes
    BF = ml_dtypes.bfloat16
    hst = np.ascontiguousarray(np.asarray(hidden_states, dtype=np.float32)
                               .reshape(TT, D).T).astype(BF)
    cost = np.asarray(cos_table, dtype=np.float32).reshape(TT, HD).T
    sint = np.asarray(sin_table, dtype=np.float32).reshape(TT, HD).T.copy()
    sint[0:HD // 2] *= -1.0     # sign of rotate-half folded into the table
    cs = np.ascontiguousarray(np.stack([cost, sint], axis=1)
                              .reshape(HD, 2 * TT))
    Wq = np.asarray(Wq, dtype=np.float32).astype(BF)
    Wk = np.asarray(Wk, dtype=np.float32).astype(BF)
    Wv = np.asarray(Wv, dtype=np.float32).astype(BF)
    Wo = np.asarray(Wo, dtype=np.float32).astype(BF)
    ident = np.eye(P, dtype=np.float32)
    ones = np.ones((P, P), dtype=np.float32)
    cbf = np.concatenate([ident, ones], axis=1).astype(BF)
    maps = []
    for c in range(NCORES):
        wkv = np.stack([Wk[:, c * HD:(c + 1) * HD],
                        Wv[:, c * HD:(c + 1) * HD]], axis=1)
        maps.append({
            "hst": hst,
            "cs": cs,
            "wq": np.ascontiguousarray(Wq[:, c * MQ:(c + 1) * MQ]),
            "wkv": np.ascontiguousarray(wkv.reshape(D, 2 * HD)),
            "wo": np.ascontiguousarray(Wo[c * MQ:(c + 1) * MQ, :]),
            "cbf": cbf,
        })
    return maps


# inputs identical on every core: sent once and broadcast by shard_map
_REPLICATED = {"hst", "cs", "cbf"}


def _get_runner(nbody=1):
    """Build the 8-core SPMD executable once (mirrors the multi-core branch
    of bass2jax.run_bass_via_pjrt, but cached so repeat calls don't re-jit
    or re-compile the NEFF).  Replicated inputs ship once; the zero output
    buffers the NEFF writes into are created on-device."""
    key = ("runner", nbody)
    if key in _CACHE:
        return _CACHE[key]
    import jax
    from jax.sharding import Mesh, PartitionSpec
    from jax.experimental.shard_map import shard_map
    import concourse.mybir as mybir
    from concourse import bass2jax

    nc = _get_nc(nbody)
    bass2jax.install_neuronx_cc_hook()

    part_name = nc.partition_id_tensor.name if nc.partition_id_tensor else None
    in_names, out_names, out_avals, zero_outs = [], [], [], []
    for alloc in nc.m.functions[0].allocations:
        if not isinstance(alloc, mybir.MemoryLocationSet):
            continue
        name = alloc.memorylocations[0].name
        if alloc.kind == "ExternalInput":
            if name != part_name:
                in_names.append(name)
        elif alloc.kind == "ExternalOutput":
            out_names.append(name)
            shape = tuple(alloc.tensor_shape)
            dtype = mybir.dt.np(alloc.dtype)
            out_avals.append(jax.core.ShapedArray(shape, dtype))
            zero_outs.append(np.zeros(shape, dtype))
    n_params = len(in_names)
    all_names = in_names + out_names
    if part_name is not None:
        all_names = all_names + [part_name]

    def _body(*args):
        operands = list(args)
        if part_name is not None:
            operands.append(bass2jax.partition_id_tensor())
        outs = bass2jax._bass_exec_p.bind(
            *operands,
            out_avals=tuple(out_avals),
            in_names=tuple(all_names),
            out_names=tuple(out_names),
            lowering_input_output_aliases=(),
            sim_require_finite=True,
            sim_require_nnan=True,
            nc=nc,
        )
        return tuple(outs)

    devices = jax.devices()[:NCORES]
    assert len(devices) == NCORES, (
        f"need {NCORES} NeuronCores, jax.devices() shows {len(jax.devices())}")
    mesh = Mesh(np.asarray(devices), ("core",))
    in_specs = tuple(PartitionSpec() if n in _REPLICATED
                     else PartitionSpec("core") for n in in_names) \
        + (PartitionSpec("core"),) * len(out_names)
    sharded = jax.jit(
        shard_map(_body, mesh=mesh,
                  in_specs=in_specs,
                  out_specs=(PartitionSpec("core"),) * len(out_names),
                  check_rep=False),
        keep_unused=True,
    )
    runner = (sharded, mesh, in_names, out_names, out_avals, zero_outs)
    _CACHE[key] = runner
    return runner


def _concat_inputs(maps):
    sharded, mesh, in_names, out_names, out_avals, zero_outs = _get_runner()
    concat_in = [maps[0][n] if n in _REPLICATED
                 else np.concatenate([maps[c][n] for c in range(NCORES)], axis=0)
                 for n in in_names]
    concat_zeros = [np.zeros((NCORES * z.shape[0], *z.shape[1:]), z.dtype)
                    for z in zero_outs]
    return concat_in + concat_zeros


def _run(maps):
    sharded, mesh, in_names, out_names, out_avals, zero_outs = _get_runner()
    out_arrs = sharded(*_concat_inputs(maps))
    return [np.asarray(out_arrs[0]).reshape(NCORES, *out_avals[0].shape)[c]
            for c in range(NCORES)]


def kernel(hidden_states, cos_table, sin_table, Wq, Wk, Wv, Wo):
    maps = _in_maps(hidden_states, cos_table, sin_table, Wq, Wk, Wv, Wo)
    parts = np.stack(_run(maps))
    out = parts.astype(np.float32).sum(axis=0)
    return out.reshape(B, S, D)
